# revision 8
# baseline (speedup 1.0000x reference)
"""Multi-head self-attention on 8 Trainium2 NeuronCores.

Problem: x(2,2048,1024), 16 heads of 64, fp32 reference. Sharding: batch (2) x
head-groups (4 groups of 4 heads). Each core computes Q/K/V projections for its
256 head-dims, attention for its 4 heads, and a partial out-projection (its 256
rows of Wo). Host sums the 4 group-partials per batch (the tensor-parallel
all-reduce) and adds bo.

Kernel layout (per core), v3 — ScalarE(exp)-bound pipeline, projections fused
into the tick stream:
  All matmul operands fp16 (PSUM accumulation fp32; softmax stats fp32).
  xT [1024,2048]; QT/KT [256,2048] (head pair per 128-partition tile),
  V natural [2048,256] with a ones column per head (V_aug).
  Only the pair-0 Q/K projection runs before the first score tick; the V
  projection and pair-1 Q/K run as PE filler work interleaved into ticks 0-3,
  so ScalarE (the bottleneck: 16 exp insts/tick ~ 16.6us vs PE ~10-14us)
  starts ~30us earlier than a serial proj phase would allow.
  scoresT[t,s] per head-PAIR with PE row-group packing (two K=64 matmuls in
  disjoint row groups stream concurrently); exp(x/8) on ScalarE only.
  attnV: ctxT_aug[65,s] = V_aug^T @ attnT, one tick behind scores.
  Normalization reads ctx + denominators straight from PSUM (DVE reciprocal
  on the [1,512] denom row, gpsimd partition_broadcast, DVE multiply).
  out partial = ctx @ Wo; PSUM evacuation split across DVE and Pool so
  ScalarE stays pure-exp.
"""

import sys

sys.path.insert(0, "/opt/trn_rl_repo")

import numpy as np

import concourse.bacc as bacc
import concourse.mybir as mybir
import concourse.tile as tile
from concourse import bass_utils

N_CORES = 8
B, S, D = 2, 2048, 1024
H_LOC = 4          # heads per core
DH = 64            # head dim
DG = H_LOC * DH    # 256 group dims per core
KC = D // 128      # 8 contraction chunks over D
ST = S // 128      # 16 s/t tiles
SC = S // 512      # 4 512-wide s chunks
MT = DG // 128     # 2 m-tiles of group dims

F32 = mybir.dt.float32
F16 = mybir.dt.float16


def _build_program(reps=1, num_devices=N_CORES):
    nc = bacc.Bacc("TRN2", target_bir_lowering=False, debug=False,
                   num_devices=num_devices)

    xT_d = nc.dram_tensor("xT", [KC, 128, S], F16, kind="ExternalInput")
    wq_d = nc.dram_tensor("wq", [KC, 128, DG], F16, kind="ExternalInput")
    wk_d = nc.dram_tensor("wk", [KC, 128, DG], F16, kind="ExternalInput")
    wv_d = nc.dram_tensor("wv", [KC, 128, DG], F16, kind="ExternalInput")
    bq_d = nc.dram_tensor("bq", [MT, 128, 1], F32, kind="ExternalInput")
    bk_d = nc.dram_tensor("bk", [MT, 128, 1], F32, kind="ExternalInput")
    bv_d = nc.dram_tensor("bv", [1, DG], F32, kind="ExternalInput")
    wo_d = nc.dram_tensor("wo", [MT, 128, D], F16, kind="ExternalInput")
    out_d = nc.dram_tensor("out", [S, D], F32, kind="ExternalOutput")

    with tile.TileContext(nc) as tc:
      for _rep in range(reps):
        with (
            tc.tile_pool(name="wpool", bufs=1) as wpool,
            tc.tile_pool(name="mpool", bufs=1) as mpool,
            tc.tile_pool(name="xpool", bufs=1) as xpool,
            tc.tile_pool(name="apool", bufs=1) as apool,
            tc.tile_pool(name="psum", bufs=4, space="PSUM") as pp,
        ):
            # ---- weights / biases ----
            wq_t = wpool.tile([128, KC, DG], F16)
            wk_t = wpool.tile([128, KC, DG], F16)
            wv_t = wpool.tile([128, KC, DG], F16)
            wo_t = wpool.tile([128, MT, D], F16)
            bq_t = wpool.tile([128, MT], F32)
            bk_t = wpool.tile([128, MT], F32)
            bv_row = wpool.tile([1, DG], F32)
            bv_bc = wpool.tile([128, H_LOC, DH], F32)
            for m in range(MT):
                nc.sync.dma_start(wo_t[:, m, :], wo_d.ap()[m])
                nc.sync.dma_start(bq_t[:, m : m + 1], bq_d.ap()[m])
                nc.sync.dma_start(bk_t[:, m : m + 1], bk_d.ap()[m])
            nc.sync.dma_start(bv_row[:], bv_d.ap())
            nc.gpsimd.partition_broadcast(bv_bc[:, :, :], bv_row[:])

            # ---- persistent intermediates (all fp16 matmul operands) ----
            qT_t = mpool.tile([128, MT, S], F16)    # [dg_row, mt, s]
            kT_t = mpool.tile([128, MT, S], F16)
            vaug = mpool.tile([128, ST, H_LOC, DH + 1], F16)
            ctxN = mpool.tile([128, MT, S], F16)    # normalized ctx^T
            nc.gpsimd.memset(vaug[:], 1.0)

            xT_t = xpool.tile([128, KC, S], F16)
            for k in range(KC):
                nc.sync.dma_start(xT_t[:, k, :], xT_d.ap()[k])
                nc.sync.dma_start(wq_t[:, k, :], wq_d.ap()[k])
                nc.sync.dma_start(wk_t[:, k, :], wk_d.ap()[k])
                nc.sync.dma_start(wv_t[:, k, :], wv_d.ap()[k])

            # ---- m0 Q/K projection (k-outer, 4 slots: PE starts on xT
            # chunk 0 while the rest stream in) ----
            ps_qks = [
                pp.tile([128, 1024], F32, tag="ps", name=f"ps_qk{sc}")
                for sc in range(SC)
            ]
            for k in range(KC):
                for sc in range(SC):
                    sl = slice(sc * 512, sc * 512 + 512)
                    nc.tensor.matmul(
                        ps_qks[sc][:, 0:512],
                        wq_t[:, k, 0:128],
                        xT_t[:, k, sl],
                        start=(k == 0), stop=(k == KC - 1),
                    )
                    nc.tensor.matmul(
                        ps_qks[sc][:, 512:1024],
                        wk_t[:, k, 0:128],
                        xT_t[:, k, sl],
                        start=(k == 0), stop=(k == KC - 1),
                    )
            for sc in range(SC):
                sl = slice(sc * 512, sc * 512 + 512)
                nc.vector.tensor_scalar_add(
                    qT_t[:, 0, sl], ps_qks[sc][:, 0:512], bq_t[:, 0:1])
                nc.vector.tensor_scalar_add(
                    kT_t[:, 0, sl], ps_qks[sc][:, 512:1024], bk_t[:, 0:1])

            # ---- filler emitters (proj work interleaved into ticks) ----
            def emit_v_filler(st, pair):
                # V projection for one s-tile, one head pair (128 dims)
                ps_v = pp.tile([128, 16, DH], F32, tag="ps", name="ps_v")
                cs = slice(pair * 128, pair * 128 + 128)
                for k in range(KC):
                    nc.tensor.matmul(
                        ps_v[:, 0:2, :],
                        xT_t[:, k, st * 128 : st * 128 + 128],
                        wv_t[:, k, cs],
                        start=(k == 0), stop=(k == KC - 1),
                    )
                # evac + bias into vaug rows for both heads of the pair
                nc.vector.tensor_add(
                    vaug[:, st, 2 * pair : 2 * pair + 2, 0:DH],
                    ps_v[:, 0:2, :],
                    bv_bc[:, 2 * pair : 2 * pair + 2, :],
                )

            def emit_m1_filler(sc, which):
                # pair-1 Q or K projection for one 512-wide s-chunk
                ssl = slice(sc * 512, sc * 512 + 512)
                ps_m = pp.tile([128, 1024], F32, tag="ps", name="ps_m")
                w_t = wq_t if which == "q" else wk_t
                for k in range(KC):
                    nc.tensor.matmul(
                        ps_m[:, 0:512],
                        w_t[:, k, 128:256],
                        xT_t[:, k, ssl],
                        start=(k == 0), stop=(k == KC - 1),
                    )
                dst = qT_t if which == "q" else kT_t
                b_t = bq_t if which == "q" else bk_t
                nc.vector.tensor_scalar_add(
                    dst[:, 1, ssl], ps_m[:, 0:512], b_t[:, 1:2])

            # ---- attention tick stream ----
            at_tiles = {}

            def emit_normalize_chunk(pair, sc, ps_c):
                # normalize one 512-wide s-chunk of both heads straight from
                # PSUM: row 64 is the denominator (ones column of V_aug)
                mt = pair
                ssl = slice(sc * 512, sc * 512 + 512)
                for i in range(2):
                    csl = slice(i * 512, i * 512 + 512)
                    rr = apool.tile([1, 512], F32, tag="rr", bufs=2,
                                    name="rr")
                    nc.vector.reciprocal(rr[:], ps_c[64:65, csl])
                    r_bc = apool.tile([64, 512], F32, tag="rbc", bufs=2,
                                      name="r_bc")
                    nc.gpsimd.partition_broadcast(r_bc[:], rr[:])
                    if i == 0:
                        nc.vector.tensor_mul(
                            ctxN[0:64, mt, ssl], ps_c[0:64, csl], r_bc[:])
                    else:
                        csh = apool.tile([64, 512], F16, tag="csh", bufs=2,
                                         name="csh")
                        nc.vector.tensor_mul(
                            csh[:], ps_c[0:64, csl], r_bc[:])
                        nc.sync.dma_start(ctxN[64:128, mt, ssl], csh[:])

            def emit_outproj(st):
                ps_o = pp.tile([128, 1024], F32, tag="ps", name="ps_o")
                # m-outer: each ctxN stationary reused for both n-chunks
                for m in range(MT):
                    for n in range(2):
                        nsl = slice(n * 512, n * 512 + 512)
                        nc.tensor.matmul(
                            ps_o[:, nsl],
                            ctxN[:, m, st * 128 : st * 128 + 128],
                            wo_t[:, m, nsl],
                            start=(m == 0), stop=(m == MT - 1),
                        )
                o_t = apool.tile([128, 1024], F32, tag="ot", bufs=4,
                                 name="o_t")
                # evacuation on DVE only (ScalarE stays pure-exp; GPSIMD
                # cannot access PSUM)
                nc.vector.tensor_copy(o_t[:], ps_o[:])
                nc.sync.dma_start(
                    out_d.ap()[st * 128 : st * 128 + 128, :], o_t[:])

            # filler schedule: V pair-0 during tick 0 (needed by attnV of
            # tick 0, which runs during tick 1), V pair-1 during tick 1,
            # pair-1 Q/K during ticks 0/2/3 (needed by tick-4 scores)
            fillers = {
                0: [("v", st, 0) for st in range(ST)]
                   + [("m1", 0, "q"), ("m1", 0, "k")],
                1: [("v", st, 1) for st in range(ST)],
                2: [("m1", 1, "q"), ("m1", 1, "k"),
                    ("m1", 2, "q"), ("m1", 2, "k")],
                3: [("m1", 3, "q"), ("m1", 3, "k")],
            }

            def emit_filler(f):
                if f[0] == "v":
                    emit_v_filler(f[1], f[2])
                else:
                    emit_m1_filler(f[1], f[2])

            ticks = [(p, sc) for p in range(2) for sc in range(SC)]
            prev = None  # (pair, sc, tiles)
            for t in range(len(ticks) + 1):
                cur_tick = ticks[t] if t < len(ticks) else None
                # ps_c accumulates the PREVIOUS tick's ctx during this one
                ps_c = (pp.tile([128, 1024], F32, tag="ps", name="ps_c")
                        if prev is not None else None)
                todo = list(fillers.get(t, []))
                cur = []
                for st in range(ST):
                    if cur_tick is not None:
                        pair, sc = cur_tick
                        ssl = slice(sc * 512, sc * 512 + 512)
                        tsl = slice(st * 128, st * 128 + 128)
                        ps_s = pp.tile([128, 1024], F32, tag="ps",
                                       name="ps_s")
                        # two K=64 matmuls in disjoint PE row groups
                        nc.tensor.matmul(
                            ps_s[:, 0:512],
                            kT_t[0:64, pair, tsl], qT_t[0:64, pair, ssl])
                        nc.tensor.matmul(
                            ps_s[:, 512:1024],
                            kT_t[64:128, pair, tsl], qT_t[64:128, pair, ssl])
                        at = apool.tile([128, 1024], F16, tag="attnT",
                                        bufs=32, name="at")
                        nc.scalar.activation(
                            at[:], ps_s[:],
                            mybir.ActivationFunctionType.Exp, scale=0.125)
                        cur.append(at)
                    if todo:
                        emit_filler(todo.pop(0))
                    if prev is not None:
                        vpair, vsc, tiles = prev
                        k = st
                        for i, h in enumerate((2 * vpair, 2 * vpair + 1)):
                            csl = slice(i * 512, i * 512 + 512)
                            nc.tensor.matmul(
                                ps_c[0:65, csl],
                                vaug[:, k, h, :],
                                tiles[k][:, csl],
                                start=(k == 0), stop=(k == ST - 1),
                            )
                for f in todo:
                    emit_filler(f)
                if prev is not None:
                    vpair, vsc, _ = prev
                    emit_normalize_chunk(vpair, vsc, ps_c)
                    if vpair == 1:
                        # this s-range of ctxN is complete for all heads:
                        # out-projection chases the last pair's attnV
                        for st_o in range(vsc * 4, vsc * 4 + 4):
                            emit_outproj(st_o)
                prev = ((cur_tick[0], cur_tick[1], cur)
                        if cur_tick else None)

    nc.compile()
    return nc


_CACHE = {}


def _get_program():
    if "nc" not in _CACHE:
        _CACHE["nc"] = _build_program()
    return _CACHE["nc"]


def _shard_inputs(x, Wq, bq, Wk, bk, Wv, bv, Wo):
    xT16 = [
        np.ascontiguousarray(x[b].T).astype(np.float16).reshape(KC, 128, S)
        for b in range(B)
    ]
    in_maps = []
    for c in range(N_CORES):
        b, g = c // 4, c % 4
        gs = slice(g * DG, g * DG + DG)
        in_maps.append({
            "xT": xT16[b],
            "wq": np.ascontiguousarray(Wq[:, gs]).astype(np.float16).reshape(KC, 128, DG),
            "wk": np.ascontiguousarray(Wk[:, gs]).astype(np.float16).reshape(KC, 128, DG),
            "wv": np.ascontiguousarray(Wv[:, gs]).astype(np.float16).reshape(KC, 128, DG),
            "bq": np.ascontiguousarray(bq[gs]).astype(np.float32).reshape(MT, 128, 1),
            "bk": np.ascontiguousarray(bk[gs]).astype(np.float32).reshape(MT, 128, 1),
            "bv": np.ascontiguousarray(bv[gs]).astype(np.float32).reshape(1, DG),
            "wo": np.ascontiguousarray(Wo[gs, :]).astype(np.float16).reshape(MT, 128, D),
        })
    return in_maps


def kernel(x, Wq, bq, Wk, bk, Wv, bv, Wo, bo, _trace=False, _trace_kwargs=None):
    x = np.asarray(x, dtype=np.float32)
    Wq, bq = np.asarray(Wq, np.float32), np.asarray(bq, np.float32)
    Wk, bk = np.asarray(Wk, np.float32), np.asarray(bk, np.float32)
    Wv, bv = np.asarray(Wv, np.float32), np.asarray(bv, np.float32)
    Wo, bo = np.asarray(Wo, np.float32), np.asarray(bo, np.float32)

    nc = _get_program()
    in_maps = _shard_inputs(x, Wq, bq, Wk, bk, Wv, bv, Wo)
    kwargs = {}
    if _trace:
        kwargs["trace"] = True
        kwargs.update(_trace_kwargs or {})
    res = bass_utils.run_bass_kernel_spmd(
        nc, in_maps, core_ids=list(range(N_CORES)), **kwargs)

    out = np.zeros((B, S, D), dtype=np.float32)
    for c in range(N_CORES):
        out[c // 4] += res.results[c]["out"]
    out += bo
    if _trace:
        kernel.last_result = res
    return out
